# revision 1
# baseline (speedup 1.0000x reference)
import sys

sys.path.insert(0, "/opt/trn_rl_repo")

import numpy as np

import concourse.bass as bass
import concourse.tile as tile
from concourse import bacc, mybir
from concourse._compat import get_trn_type

EPS = 1e-6

BS, NSEQ, NB, NC_, ML = 32, 24, 196, 196, 6
BPC = 4            # batches per core
NCORES = 8
P = 112            # partition chunk for (b,i) rows: 4*196=784 = 7*112
NCHUNK = 7
EBLK = 8           # e-rows per scatter block: f = 8*196 = 1568
NEB = 3            # 24 = 3*8
FB = EBLK * NB     # 1568
EM = NSEQ * NB     # 4704
HALF = 98          # m-half for C^T chunks: 196 = 2*98
NKT = NSEQ * 2     # 48 C^T chunks (e, half)
ROWS = BPC * NB    # 784


def _host_prep(trav, adj, ent, spo, ctx, roi_cls, roi_mask, w_child):
    """Per-core (4-batch slice) host index/mask prep. Only int-derived
    index/mask/selector tensors and input reshapes — no float math on the
    attention data."""
    import ml_dtypes
    f32, i16 = np.float32, np.int16
    kcls = (roi_cls != -1).astype(f32)                     # [4, 196]

    rows_b = (np.arange(ROWS) // NB).astype(np.int64)
    rows_i = (np.arange(ROWS) % NB).astype(np.int64)
    ctx_rows = ctx[rows_b, rows_i]                         # [784, 196]

    order = np.argsort(ctx_rows, axis=1, kind="stable")
    rank = np.argsort(order, axis=1, kind="stable")
    m_sorted = np.take_along_axis(ctx_rows, order, axis=1)
    first = np.ones_like(m_sorted, dtype=bool)
    first[:, 1:] = m_sorted[:, 1:] != m_sorted[:, :-1]
    last = np.ones_like(m_sorted, dtype=bool)
    last[:, :-1] = m_sorted[:, :-1] != m_sorted[:, 1:]

    off = (np.arange(EBLK) * NB).astype(np.int64)
    idx_sig = (rank[:, None, :] + off[None, :, None]).reshape(ROWS, FB)
    segm = np.where(first, 0.0, 1.0).astype(np.float32)
    segm = np.broadcast_to(segm[:, None, :], (ROWS, EBLK, NB)).reshape(ROWS, FB)
    bnd = np.where(last, m_sorted, -1).astype(np.int64)
    idx_bnd = np.where(
        bnd[:, None, :] >= 0, bnd[:, None, :] + off[None, :, None], -1
    ).reshape(ROWS, FB)

    def chunks(a):  # [784, F] -> [112, 7*F]
        return np.concatenate([a[c * P:(c + 1) * P] for c in range(NCHUNK)], axis=1)

    idx_sig_t = np.ascontiguousarray(chunks(idx_sig).astype(i16))
    idx_bnd_t = np.ascontiguousarray(chunks(idx_bnd).astype(i16))
    segm_t = np.ascontiguousarray(chunks(segm).astype(ml_dtypes.bfloat16))
    kcls_chunk = np.ascontiguousarray(
        kcls[rows_b, rows_i].reshape(NCHUNK, P).T.astype(f32))   # [112, 7]

    Mt = np.zeros((128, ML * NSEQ), dtype=f32)
    sel1 = np.zeros((128, ML * BPC), dtype=f32)
    sel2 = np.zeros((BPC, ML * 128), dtype=f32)
    w_rows = np.zeros((BPC, ML * NB), dtype=f32)
    eps4 = np.zeros((BPC, ML), dtype=f32)
    for t in range(ML):
        for b in range(BPC):
            p_raw = int(trav[b, t])
            p = max(p_raw, 0)
            edges = adj[b, p]
            cm = (edges >= 0) & (p_raw >= 0)
            ec = np.maximum(edges, 0)
            nch = int(cm.sum())
            for j in range(NSEQ):
                if cm[j]:
                    Mt[b * 32 + j, t * NSEQ + int(ec[j])] = 1.0
            sel1[b * 32 + p, t * BPC + b] = 1.0
            if nch > 0 and p_raw >= 0:
                sel2[b, t * 128 + b * 32 + p] = 1.0
            w_rows[b, t * NB:(t + 1) * NB] = w_child[b, p]
            eps4[b, t] = max(nch, 1) * EPS

    ea0 = np.zeros((128, NB), dtype=f32)
    kclsr = np.zeros((128, NB), dtype=f32)
    for b in range(BPC):
        ea0[b * 32:b * 32 + NSEQ] = ent[b]
        kclsr[b * 32:b * 32 + NSEQ] = kcls[b][None, :]

    return {
        "spo": np.ascontiguousarray(spo.astype(f32).transpose(0, 2, 1, 3)),
        "roi": np.ascontiguousarray(roi_mask.astype(f32)),
        "idx_sig": idx_sig_t,
        "idx_bnd": idx_bnd_t,
        "segm": segm_t,
        "kcls_chunk": kcls_chunk,
        "Mt": Mt, "sel1": sel1, "sel2": sel2,
        "w_rows": w_rows, "eps4": eps4,
        "ea0": ea0, "kclsr": kclsr,
        "kcls4": kcls.astype(f32),
        "maskpos": kcls.astype(f32),
        "mask_m1": (kcls - 1.0).astype(f32),
        "ident": np.eye(P, dtype=ml_dtypes.bfloat16),
    }


def _row_ranges(c):
    """(b, i0, i1, q0) sub-ranges of chunk c at batch boundaries."""
    r0, r1 = c * P, (c + 1) * P
    out = []
    r = r0
    while r < r1:
        b = r // NB
        i0 = r % NB
        i1 = min(NB, i0 + (r1 - r))
        out.append((b, i0, i1, r - r0))
        r += i1 - i0
    return out


def build_bass():
    f32 = mybir.dt.float32
    bf16 = mybir.dt.bfloat16
    i16 = mybir.dt.int16
    nc = bacc.Bacc(get_trn_type() or "TRN2", target_bir_lowering=False)

    spo_d = nc.dram_tensor("spo", (BPC, NB, NSEQ, NC_), f32, kind="ExternalInput")
    roi_d = nc.dram_tensor("roi", (BPC, NB, NC_), f32, kind="ExternalInput")
    sig_d = nc.dram_tensor("idx_sig", (P, NCHUNK * FB), i16, kind="ExternalInput")
    bnd_d = nc.dram_tensor("idx_bnd", (P, NCHUNK * FB), i16, kind="ExternalInput")
    segm_d = nc.dram_tensor("segm", (P, NCHUNK * FB), bf16, kind="ExternalInput")
    kch_d = nc.dram_tensor("kcls_chunk", (P, NCHUNK), f32, kind="ExternalInput")
    Mt_d = nc.dram_tensor("Mt", (128, ML * NSEQ), f32, kind="ExternalInput")
    sel1_d = nc.dram_tensor("sel1", (128, ML * BPC), f32, kind="ExternalInput")
    sel2_d = nc.dram_tensor("sel2", (BPC, ML * 128), f32, kind="ExternalInput")
    wr_d = nc.dram_tensor("w_rows", (BPC, ML * NB), f32, kind="ExternalInput")
    eps_d = nc.dram_tensor("eps4", (BPC, ML), f32, kind="ExternalInput")
    ea0_d = nc.dram_tensor("ea0", (128, NB), f32, kind="ExternalInput")
    kclsr_d = nc.dram_tensor("kclsr", (128, NB), f32, kind="ExternalInput")
    kcls4_d = nc.dram_tensor("kcls4", (BPC, NB), f32, kind="ExternalInput")
    mpos_d = nc.dram_tensor("maskpos", (BPC, NB), f32, kind="ExternalInput")
    mm1_d = nc.dram_tensor("mask_m1", (BPC, NB), f32, kind="ExternalInput")
    id_d = nc.dram_tensor("ident", (P, P), bf16, kind="ExternalInput")
    out_d = nc.dram_tensor("ea_out", (128, NB), f32, kind="ExternalOutput")

    with tile.TileContext(nc) as tc:
        with (
            tc.tile_pool(name="persist", bufs=1) as pp,
            tc.tile_pool(name="stage", bufs=2) as sp,
            tc.tile_pool(name="work", bufs=2) as wp,
            tc.tile_pool(name="small", bufs=2) as mp,
            tc.tile_pool(name="psA", bufs=2, space="PSUM") as psA,
            tc.tile_pool(name="psB", bufs=1, space="PSUM") as psB,
        ):
            # ---- persistent tiles ----
            CT = pp.tile([HALF, NKT * ROWS], bf16, tag="CT")
            ea = pp.tile([128, NB], f32, tag="ea")
            eam = pp.tile([128, NB], f32, tag="eam")
            kch = pp.tile([P, NCHUNK], f32, tag="kch")
            Mt = pp.tile([128, ML * NSEQ], f32, tag="Mt")
            sel1 = pp.tile([128, ML * BPC], f32, tag="sel1")
            sel2 = pp.tile([BPC, ML * 128], f32, tag="sel2")
            wr = pp.tile([BPC, ML * NB], f32, tag="wr")
            eps4 = pp.tile([BPC, ML], f32, tag="eps4")
            kclsr = pp.tile([128, NB], f32, tag="kclsr")
            kcls4 = pp.tile([BPC, NB], f32, tag="kcls4")
            mpos = pp.tile([BPC, NB], f32, tag="mpos")
            mm1 = pp.tile([BPC, NB], f32, tag="mm1")
            ident = pp.tile([P, P], bf16, tag="ident")
            ones4 = pp.tile([HALF, BPC], f32, tag="ones4")
            acc = pp.tile([HALF, ROWS], f32, tag="acc")

            for dst, src in [
                (kch, kch_d), (Mt, Mt_d), (sel1, sel1_d), (sel2, sel2_d),
                (wr, wr_d), (eps4, eps_d), (ea, ea0_d), (kclsr, kclsr_d),
                (kcls4, kcls4_d), (mpos, mpos_d), (mm1, mm1_d), (ident, id_d),
            ]:
                nc.sync.dma_start(dst[:], src[:])
            nc.vector.tensor_mul(eam[:], ea[:], kclsr[:])
            nc.vector.memset(ones4[:], 1.0)

            # ---- per chunk: spo3 -> scatter -> scan -> extract -> transpose ----
            for c in range(NCHUNK):
                st = sp.tile([P, NSEQ, NC_], f32, tag="spost")
                for (b, i0, i1, q0) in _row_ranges(c):
                    nc.sync.dma_start(
                        st[q0:q0 + (i1 - i0), :, :],
                        spo_d[b, i0:i1, :, :],
                    )
                rt = sp.tile([P, NC_], f32, tag="roist")
                for (b, i0, i1, q0) in _row_ranges(c):
                    nc.sync.dma_start(rt[q0:q0 + (i1 - i0), :], roi_d[b, i0:i1, :])
                w3c = wp.tile([P, NC_], f32, tag="w3c")
                nc.vector.tensor_mul(w3c[:], rt[:], rt[:])
                nc.vector.tensor_mul(w3c[:], w3c[:], rt[:])
                nc.vector.tensor_scalar_mul(w3c[:], w3c[:], kch[:, c:c + 1])
                sp3c = wp.tile([P, EM], bf16, tag="sp3c")
                w3b = w3c[:].unsqueeze(1).broadcast_to((P, NSEQ, NC_))
                nc.vector.tensor_mul(sp3c[:].rearrange("p (e c) -> p e c", e=NSEQ),
                                     st[:], w3b)
                sigc = wp.tile([P, FB], i16, tag="sigc")
                bndc = wp.tile([P, FB], i16, tag="bndc")
                segc = wp.tile([P, FB], bf16, tag="segc")
                Cmc = wp.tile([P, EM], bf16, tag="Cmc")
                for e in range(NEB):
                    fb0 = e * FB
                    if e == 0:
                        nc.sync.dma_start(sigc[:], sig_d[:, c * FB:(c + 1) * FB])
                        nc.sync.dma_start(bndc[:], bnd_d[:, c * FB:(c + 1) * FB])
                        nc.sync.dma_start(segc[:], segm_d[:, c * FB:(c + 1) * FB])
                    srt = wp.tile([P, FB], bf16, tag="sorted")
                    nc.gpsimd.local_scatter(
                        srt[:], sp3c[:, fb0:fb0 + FB], sigc[:],
                        channels=P, num_elems=FB, num_idxs=FB,
                    )
                    scn = wp.tile([P, FB], bf16, tag="scan")
                    nc.vector.tensor_tensor_scan(
                        scn[:], segc[:], srt[:], 0.0,
                        op0=mybir.AluOpType.mult, op1=mybir.AluOpType.add,
                    )
                    nc.gpsimd.local_scatter(
                        Cmc[:, fb0:fb0 + FB], scn[:], bndc[:],
                        channels=P, num_elems=FB, num_idxs=FB,
                    )
                for g in range(NKT // 4):
                    pt4 = psA.tile([HALF, 4, P], bf16, tag="tp")
                    for j in range(4):
                        s = g * 4 + j
                        nc.tensor.transpose(
                            pt4[:, j, :], Cmc[:, s * HALF:(s + 1) * HALF],
                            ident[:])
                    dst = (CT[:, 4 * g * ROWS: 4 * (g + 1) * ROWS]
                           .rearrange("p (s r) -> p s r", s=4)
                           [:, :, c * P:(c + 1) * P])
                    nc.scalar.copy(dst, pt4[:])

            # ---- 6 sequential steps ----
            for t in range(ML):
                a4 = [mp.tile([HALF, NSEQ, BPC], bf16, tag=f"a4_{h}",
                              name=f"a4_{h}") for h in range(2)]
                for h in range(2):
                    for b in range(BPC):
                        aps = psA.tile([HALF, NSEQ], f32, tag="aps")
                        nc.tensor.matmul(
                            aps[:],
                            eam[b * 32:b * 32 + NSEQ, h * HALF:(h + 1) * HALF],
                            Mt[b * 32:b * 32 + NSEQ, t * NSEQ:(t + 1) * NSEQ],
                            start=True, stop=True,
                            tile_position=(b * 32, 0),
                        )
                        nc.scalar.copy(a4[h][:, :, b], aps[:])
                KPE = 34
                rps = [psB.tile([BPC, 2 * NB], f32, tag=f"rps{nb}",
                                name=f"rps{nb}") for nb in range(2)]
                for k in range(NKT):
                    e, h = k // 2, k % 2
                    if k < KPE:
                        for nb in range(2):
                            nc.tensor.matmul(
                                rps[nb][:],
                                a4[h][:, e, :],
                                CT[:, k * ROWS + nb * 2 * NB: k * ROWS + (nb + 1) * 2 * NB],
                                start=(k == 0), stop=False,
                            )
                    else:
                        for b in range(BPC):
                            nc.vector.scalar_tensor_tensor(
                                acc[:, b * NB:(b + 1) * NB],
                                CT[:, k * ROWS + b * NB: k * ROWS + (b + 1) * NB],
                                a4[h][:, e, b:b + 1],
                                acc[:, b * NB:(b + 1) * NB],
                                op0=mybir.AluOpType.mult,
                                op1=(mybir.AluOpType.add if k > KPE
                                     else mybir.AluOpType.bypass),
                            )
                for nb in range(2):
                    nc.tensor.matmul(
                        rps[nb][:], ones4[:],
                        acc[:, nb * 2 * NB:(nb + 1) * 2 * NB],
                        start=False, stop=(nb == 1),
                    )
                r4 = mp.tile([BPC, NB], f32, tag="r4")
                for nb in range(2):
                    rsb = mp.tile([BPC, 2 * NB], f32, tag=f"rsb{nb}",
                                  name=f"rsb{nb}", bufs=1)
                    nc.vector.tensor_copy(rsb[:], rps[nb][:])
                    for b in (2 * nb, 2 * nb + 1):
                        nc.sync.dma_start(
                            r4[b:b + 1, :],
                            rsb[b:b + 1, (b % 2) * NB:(b % 2) * NB + NB])
                nc.vector.tensor_scalar_add(r4[:], r4[:], eps4[:, t:t + 1])
                sps = psB.tile([BPC, NB], f32, tag="sps")
                nc.tensor.matmul(sps[:], sel1[:, t * BPC:(t + 1) * BPC], ea[:],
                                 start=True, stop=True)
                srow = mp.tile([BPC, NB], f32, tag="srow")
                nc.vector.tensor_copy(srow[:], sps[:])
                upd = mp.tile([BPC, NB], f32, tag="upd")
                nc.vector.tensor_mul(upd[:], r4[:], wr[:, t * NB:(t + 1) * NB])
                nc.vector.tensor_add(upd[:], upd[:], srow[:])
                nrm = mp.tile([BPC, 1], f32, tag="nrm")
                nc.vector.tensor_reduce(nrm[:], upd[:], axis=mybir.AxisListType.X,
                                        op=mybir.AluOpType.max,
                                        apply_absolute_value=True)
                nc.vector.tensor_scalar_max(nrm[:], nrm[:], 1.0)
                rec = mp.tile([BPC, 1], f32, tag="rec")
                nc.vector.reciprocal(rec[:], nrm[:])
                nc.vector.tensor_scalar_mul(upd[:], upd[:], rec[:])
                nc.vector.tensor_mul(upd[:], upd[:], mpos[:])
                nc.vector.tensor_add(upd[:], upd[:], mm1[:])
                dd = mp.tile([BPC, 2 * NB], f32, tag="dd", bufs=1)
                nc.vector.tensor_sub(dd[:, :NB], upd[:], srow[:])
                nc.vector.tensor_mul(dd[:, NB:], dd[:, :NB], kcls4[:])
                wps = psB.tile([128, 2 * NB], f32, tag="wps")
                nc.tensor.matmul(wps[:], sel2[:, t * 128:(t + 1) * 128], dd[:],
                                 start=True, stop=True)
                nc.vector.tensor_add(ea[:], ea[:], wps[:, :NB])
                nc.vector.tensor_add(eam[:], eam[:], wps[:, NB:])

            nc.sync.dma_start(out_d[:], ea[:])

    nc.compile()
    return nc


_NC_CACHE = None


def kernel(traversal_lists, adj_matrices, ent_attn, spo_attn,
           ctx_idx_adjusted, roi_cls, roi_mask, weight_on_children):
    global _NC_CACHE
    from concourse.bass_utils import run_bass_kernel_spmd

    in_maps = []
    for k in range(NCORES):
        s = slice(k * BPC, (k + 1) * BPC)
        in_maps.append(_host_prep(
            np.asarray(traversal_lists[s]), np.asarray(adj_matrices[s]),
            np.asarray(ent_attn[s]), np.asarray(spo_attn[s]),
            np.asarray(ctx_idx_adjusted[s]), np.asarray(roi_cls[s]),
            np.asarray(roi_mask[s]), np.asarray(weight_on_children[s]),
        ))
    if _NC_CACHE is None:
        _NC_CACHE = build_bass()
    res = run_bass_kernel_spmd(_NC_CACHE, in_maps, core_ids=list(range(NCORES)))
    out = np.empty((BS, NSEQ, NB), dtype=np.float32)
    for k in range(NCORES):
        r = res.results[k]["ea_out"]
        for b in range(BPC):
            out[k * BPC + b] = r[b * 32:b * 32 + NSEQ]
    return out



# revision 7
# speedup vs baseline: 4.4688x; 4.4688x over previous
import sys

sys.path.insert(0, "/opt/trn_rl_repo")

import numpy as np

import concourse.bass as bass
import concourse.tile as tile
from concourse import bacc, mybir
from concourse._compat import get_trn_type

EPS = 1e-6

BS, NSEQ, NB, NC_, ML = 32, 24, 196, 196, 6
BPC = 4            # batches per core
NCORES = 8
P = 112            # partition chunk for (b,i) rows: 4*196=784 = 7*112
NCHUNK = 7
EBLK = 8           # e-rows per scatter block: f = 8*196 = 1568
NEB = 3            # 24 = 3*8
FB = EBLK * NB     # 1568
EM = NSEQ * NB     # 4704
HALF = 98          # m-half for C^T chunks: 196 = 2*98
NKT = NSEQ * 2     # 48 C^T chunks (e, half)
ROWS = BPC * NB    # 784
INVALID = -8192    # idx_bnd invalid marker; stays negative after +e*196


def _host_prep(trav, adj, ent, spo, ctx, roi_cls, roi_mask, w_child):
    """Per-core (4-batch slice) host index/mask prep. Only int-derived
    index/mask/selector tensors and input reshapes/dtype casts — no float
    math on the attention data."""
    import ml_dtypes
    f32, i16, bf16 = np.float32, np.int16, ml_dtypes.bfloat16
    f8 = ml_dtypes.float8_e4m3
    kcls = (roi_cls != -1).astype(f32)                     # [4, 196]

    rows_b = (np.arange(ROWS) // NB).astype(np.int64)
    rows_i = (np.arange(ROWS) % NB).astype(np.int64)
    ctx_rows = ctx[rows_b, rows_i]                         # [784, 196]

    order = np.argsort(ctx_rows, axis=1, kind="stable")
    rank = np.argsort(order, axis=1, kind="stable")
    m_sorted = np.take_along_axis(ctx_rows, order, axis=1)
    first = np.ones_like(m_sorted, dtype=bool)
    first[:, 1:] = m_sorted[:, 1:] != m_sorted[:, :-1]
    last = np.ones_like(m_sorted, dtype=bool)
    last[:, :-1] = m_sorted[:, :-1] != m_sorted[:, 1:]

    segf = np.where(first, 0.0, 1.0).astype(np.float32)    # [784, 196]
    bnd = np.where(last, m_sorted, INVALID)                # [784, 196]

    def chunks(a):  # [784, F] -> [112, 7*F]
        return np.concatenate([a[c * P:(c + 1) * P] for c in range(NCHUNK)], axis=1)

    rank_t = np.ascontiguousarray(chunks(rank).astype(i16))
    bnd_t = np.ascontiguousarray(chunks(bnd).astype(i16))
    segf_t = np.ascontiguousarray(chunks(segf).astype(bf16))
    kcls_chunk = np.ascontiguousarray(
        kcls[rows_b, rows_i].reshape(NCHUNK, P).T.astype(f32))   # [112, 7]

    Mt = np.zeros((128, ML * NSEQ), dtype=bf16)
    sel1 = np.zeros((128, ML * BPC), dtype=f32)
    sel2 = np.zeros((BPC, ML * 128), dtype=f32)
    w_rows = np.zeros((BPC, ML * NB), dtype=bf16)
    eps4 = np.zeros((BPC, ML), dtype=f32)
    for t in range(ML):
        for b in range(BPC):
            p_raw = int(trav[b, t])
            p = max(p_raw, 0)
            edges = adj[b, p]
            cm = (edges >= 0) & (p_raw >= 0)
            ec = np.maximum(edges, 0)
            nch = int(cm.sum())
            for j in range(NSEQ):
                if cm[j]:
                    Mt[b * 32 + j, t * NSEQ + int(ec[j])] = 1.0
            sel1[b * 32 + p, t * BPC + b] = 1.0
            if nch > 0 and p_raw >= 0:
                sel2[b, t * 128 + b * 32 + p] = 1.0
            w_rows[b, t * NB:(t + 1) * NB] = w_child[b, p].astype(bf16)
            eps4[b, t] = max(nch, 1) * EPS

    ea0 = np.ascontiguousarray(ent.reshape(BPC * NSEQ, NB).astype(f32))
    kclsr = np.zeros((128, NB), dtype=bf16)
    for b in range(BPC):
        kclsr[b * 32:b * 32 + NSEQ] = kcls[b][None, :].astype(bf16)

    return {
        "spo": np.ascontiguousarray(spo.transpose(0, 2, 1, 3)).astype(f8),
        "roi": roi_mask.astype(bf16),
        "rank_c": rank_t,
        "bnd_c": bnd_t,
        "segf_c": segf_t,
        "kcls_chunk": kcls_chunk,
        "Mt": Mt, "sel1": sel1, "sel2": sel2,
        "w_rows": w_rows, "eps4": eps4,
        "ea0": ea0, "kclsr": kclsr,
        "kcls4": kcls.astype(f32),
        "ident": np.eye(P, dtype=bf16),
    }


def _row_ranges(c):
    """(b, i0, i1, q0) sub-ranges of chunk c at batch boundaries."""
    r0, r1 = c * P, (c + 1) * P
    out = []
    r = r0
    while r < r1:
        b = r // NB
        i0 = r % NB
        i1 = min(NB, i0 + (r1 - r))
        out.append((b, i0, i1, r - r0))
        r += i1 - i0
    return out


def build_bass():
    f32 = mybir.dt.float32
    bf16 = mybir.dt.bfloat16
    i16 = mybir.dt.int16
    f8 = mybir.dt.float8e4
    nc = bacc.Bacc(get_trn_type() or "TRN2", target_bir_lowering=False)

    spo_d = nc.dram_tensor("spo", (BPC, NB, NSEQ, NC_), f8, kind="ExternalInput")
    roi_d = nc.dram_tensor("roi", (BPC, NB, NC_), bf16, kind="ExternalInput")
    rank_d = nc.dram_tensor("rank_c", (P, NCHUNK * NC_), i16, kind="ExternalInput")
    bnd_d = nc.dram_tensor("bnd_c", (P, NCHUNK * NC_), i16, kind="ExternalInput")
    segf_d = nc.dram_tensor("segf_c", (P, NCHUNK * NC_), bf16, kind="ExternalInput")
    kch_d = nc.dram_tensor("kcls_chunk", (P, NCHUNK), f32, kind="ExternalInput")
    Mt_d = nc.dram_tensor("Mt", (128, ML * NSEQ), bf16, kind="ExternalInput")
    sel1_d = nc.dram_tensor("sel1", (128, ML * BPC), f32, kind="ExternalInput")
    sel2_d = nc.dram_tensor("sel2", (BPC, ML * 128), f32, kind="ExternalInput")
    wr_d = nc.dram_tensor("w_rows", (BPC, ML * NB), bf16, kind="ExternalInput")
    eps_d = nc.dram_tensor("eps4", (BPC, ML), f32, kind="ExternalInput")
    ea0_d = nc.dram_tensor("ea0", (BPC * NSEQ, NB), f32, kind="ExternalInput")
    kclsr_d = nc.dram_tensor("kclsr", (128, NB), bf16, kind="ExternalInput")
    kcls4_d = nc.dram_tensor("kcls4", (BPC, NB), f32, kind="ExternalInput")
    id_d = nc.dram_tensor("ident", (P, P), bf16, kind="ExternalInput")
    out_d = nc.dram_tensor("ea_out", (BPC * NSEQ, NB), f32, kind="ExternalOutput")

    with tile.TileContext(nc) as tc:
        with (
            tc.tile_pool(name="persist", bufs=1) as pp,
            tc.tile_pool(name="stage", bufs=2) as sp,
            tc.tile_pool(name="work", bufs=2) as wp,
            tc.tile_pool(name="small", bufs=2) as mp,
            tc.tile_pool(name="psA", bufs=2, space="PSUM") as psA,
            tc.tile_pool(name="psB", bufs=1, space="PSUM") as psB,
        ):
            # ---- persistent tiles ----
            CT = pp.tile([HALF, NKT * ROWS], bf16, tag="CT")
            ea = pp.tile([128, NB], f32, tag="ea")
            eam = pp.tile([128, NB], bf16, tag="eam")
            kch = pp.tile([P, NCHUNK], f32, tag="kch")
            Mt = pp.tile([128, ML * NSEQ], bf16, tag="Mt")
            sel1 = pp.tile([128, ML * BPC], f32, tag="sel1")
            sel2 = pp.tile([BPC, ML * 128], f32, tag="sel2")
            wr = pp.tile([BPC, ML * NB], bf16, tag="wr")
            eps4 = pp.tile([BPC, ML], f32, tag="eps4")
            kclsr = pp.tile([128, NB], bf16, tag="kclsr")
            kcls4 = pp.tile([BPC, NB], f32, tag="kcls4")
            mm1 = pp.tile([BPC, NB], f32, tag="mm1")
            ident = pp.tile([P, P], bf16, tag="ident")
            ones4 = pp.tile([HALF, BPC], f32, tag="ones4")
            acc = pp.tile([HALF, ROWS], f32, tag="acc")

            for dst, src in [
                (kch, kch_d), (Mt, Mt_d), (sel1, sel1_d), (sel2, sel2_d),
                (wr, wr_d), (eps4, eps_d), (kclsr, kclsr_d),
                (kcls4, kcls4_d), (ident, id_d),
            ]:
                nc.sync.dma_start(dst[:], src[:])
            nc.vector.memset(ea[:], 0.0)
            for b in range(BPC):
                nc.sync.dma_start(ea[b * 32:b * 32 + NSEQ, :],
                                  ea0_d[b * NSEQ:(b + 1) * NSEQ, :])
            nc.vector.tensor_mul(eam[:], ea[:], kclsr[:])
            nc.vector.tensor_scalar_add(mm1[:], kcls4[:], -1.0)
            nc.vector.memset(ones4[:], 1.0)

            # ---- per chunk: spo3 -> scatter -> scan -> extract -> transpose ----
            for c in range(NCHUNK):
                st = sp.tile([P, NSEQ, NC_], f8, tag="spost")
                for (b, i0, i1, q0) in _row_ranges(c):
                    nc.sync.dma_start(
                        st[q0:q0 + (i1 - i0), :, :],
                        spo_d[b, i0:i1, :, :],
                    )
                rt = sp.tile([P, NC_], bf16, tag="roist")
                for (b, i0, i1, q0) in _row_ranges(c):
                    nc.sync.dma_start(rt[q0:q0 + (i1 - i0), :], roi_d[b, i0:i1, :])
                # roi is binary so roi^3 * kcls == roi * kcls
                w3c = wp.tile([P, NC_], f32, tag="w3c")
                nc.vector.tensor_scalar_mul(w3c[:], rt[:], kch[:, c:c + 1])
                sp3c = wp.tile([P, EM], bf16, tag="sp3c")
                w3b = w3c[:].unsqueeze(1).broadcast_to((P, NSEQ, NC_))
                nc.vector.tensor_mul(sp3c[:].rearrange("p (e c) -> p e c", e=NSEQ),
                                     st[:], w3b)
                # expand compact per-row idx/segment tensors to 8 e-rows
                rankc = wp.tile([P, NC_], i16, tag="rankc")
                bnd0 = wp.tile([P, NC_], i16, tag="bnd0")
                seg0 = wp.tile([P, NC_], bf16, tag="seg0")
                nc.sync.dma_start(rankc[:], rank_d[:, c * NC_:(c + 1) * NC_])
                nc.sync.dma_start(bnd0[:], bnd_d[:, c * NC_:(c + 1) * NC_])
                nc.sync.dma_start(seg0[:], segf_d[:, c * NC_:(c + 1) * NC_])
                sigc = wp.tile([P, FB], i16, tag="sigc")
                bndc = wp.tile([P, FB], i16, tag="bndc")
                segc = wp.tile([P, FB], bf16, tag="segc")
                for e in range(EBLK):
                    s = slice(e * NC_, (e + 1) * NC_)
                    nc.vector.tensor_scalar_add(sigc[:, s], rankc[:], e * NC_)
                    nc.vector.tensor_scalar_add(bndc[:, s], bnd0[:], e * NC_)
                    nc.scalar.copy(segc[:, s], seg0[:])
                Cmc = wp.tile([P, EM], bf16, tag="Cmc")
                for e in range(NEB):
                    fb0 = e * FB
                    srt = wp.tile([P, FB], bf16, tag="sorted")
                    nc.gpsimd.local_scatter(
                        srt[:], sp3c[:, fb0:fb0 + FB], sigc[:],
                        channels=P, num_elems=FB, num_idxs=FB,
                    )
                    scn = wp.tile([P, FB], bf16, tag="scan")
                    nc.vector.tensor_tensor_scan(
                        scn[:], segc[:], srt[:], 0.0,
                        op0=mybir.AluOpType.mult, op1=mybir.AluOpType.add,
                    )
                    nc.gpsimd.local_scatter(
                        Cmc[:, fb0:fb0 + FB], scn[:], bndc[:],
                        channels=P, num_elems=FB, num_idxs=FB,
                    )
                for g in range(NKT // 4):
                    pt4 = psA.tile([HALF, 4, P], bf16, tag="tp")
                    for j in range(4):
                        s = g * 4 + j
                        nc.tensor.transpose(
                            pt4[:, j, :], Cmc[:, s * HALF:(s + 1) * HALF],
                            ident[:])
                    dst = (CT[:, 4 * g * ROWS: 4 * (g + 1) * ROWS]
                           .rearrange("p (s r) -> p s r", s=4)
                           [:, :, c * P:(c + 1) * P])
                    nc.scalar.copy(dst, pt4[:])

            # ---- 6 sequential steps ----
            for t in range(ML):
                a4 = [mp.tile([HALF, NSEQ, BPC], bf16, tag=f"a4_{h}",
                              name=f"a4_{h}") for h in range(2)]
                for h in range(2):
                    for b in range(BPC):
                        aps = psA.tile([HALF, NSEQ], f32, tag="aps")
                        nc.tensor.matmul(
                            aps[:],
                            eam[b * 32:b * 32 + NSEQ, h * HALF:(h + 1) * HALF],
                            Mt[b * 32:b * 32 + NSEQ, t * NSEQ:(t + 1) * NSEQ],
                            start=True, stop=True,
                            tile_position=(b * 32, 0),
                        )
                        nc.scalar.copy(a4[h][:, :, b], aps[:])
                KPE = 34
                rps = [psB.tile([BPC, 2 * NB], f32, tag=f"rps{nb}",
                                name=f"rps{nb}") for nb in range(2)]
                for k in range(NKT):
                    e, h = k // 2, k % 2
                    if k < KPE:
                        for nb in range(2):
                            nc.tensor.matmul(
                                rps[nb][:],
                                a4[h][:, e, :],
                                CT[:, k * ROWS + nb * 2 * NB: k * ROWS + (nb + 1) * 2 * NB],
                                start=(k == 0), stop=False,
                            )
                    else:
                        for b in range(BPC):
                            nc.vector.scalar_tensor_tensor(
                                acc[:, b * NB:(b + 1) * NB],
                                CT[:, k * ROWS + b * NB: k * ROWS + (b + 1) * NB],
                                a4[h][:, e, b:b + 1],
                                acc[:, b * NB:(b + 1) * NB],
                                op0=mybir.AluOpType.mult,
                                op1=(mybir.AluOpType.add if k > KPE
                                     else mybir.AluOpType.bypass),
                            )
                for nb in range(2):
                    nc.tensor.matmul(
                        rps[nb][:], ones4[:],
                        acc[:, nb * 2 * NB:(nb + 1) * 2 * NB],
                        start=False, stop=(nb == 1),
                    )
                r4 = mp.tile([BPC, NB], f32, tag="r4")
                for nb in range(2):
                    rsb = mp.tile([BPC, 2 * NB], f32, tag=f"rsb{nb}",
                                  name=f"rsb{nb}", bufs=1)
                    nc.vector.tensor_copy(rsb[:], rps[nb][:])
                    for b in (2 * nb, 2 * nb + 1):
                        nc.sync.dma_start(
                            r4[b:b + 1, :],
                            rsb[b:b + 1, (b % 2) * NB:(b % 2) * NB + NB])
                nc.vector.tensor_scalar_add(r4[:], r4[:], eps4[:, t:t + 1])
                sps = psB.tile([BPC, NB], f32, tag="sps")
                nc.tensor.matmul(sps[:], sel1[:, t * BPC:(t + 1) * BPC], ea[:],
                                 start=True, stop=True)
                srow = mp.tile([BPC, NB], f32, tag="srow")
                nc.vector.tensor_copy(srow[:], sps[:])
                upd = mp.tile([BPC, NB], f32, tag="upd")
                nc.vector.tensor_mul(upd[:], r4[:], wr[:, t * NB:(t + 1) * NB])
                nc.vector.tensor_add(upd[:], upd[:], srow[:])
                nrm = mp.tile([BPC, 1], f32, tag="nrm")
                nc.vector.tensor_reduce(nrm[:], upd[:], axis=mybir.AxisListType.X,
                                        op=mybir.AluOpType.max,
                                        apply_absolute_value=True)
                nc.vector.tensor_scalar_max(nrm[:], nrm[:], 1.0)
                rec = mp.tile([BPC, 1], f32, tag="rec")
                nc.vector.reciprocal(rec[:], nrm[:])
                nc.vector.tensor_scalar_mul(upd[:], upd[:], rec[:])
                nc.vector.tensor_mul(upd[:], upd[:], kcls4[:])
                nc.vector.tensor_add(upd[:], upd[:], mm1[:])
                dd = mp.tile([BPC, 2 * NB], f32, tag="dd", bufs=1)
                nc.vector.tensor_sub(dd[:, :NB], upd[:], srow[:])
                nc.vector.tensor_mul(dd[:, NB:], dd[:, :NB], kcls4[:])
                wps = psB.tile([128, 2 * NB], f32, tag="wps")
                nc.tensor.matmul(wps[:], sel2[:, t * 128:(t + 1) * 128], dd[:],
                                 start=True, stop=True)
                nc.vector.tensor_add(ea[:], ea[:], wps[:, :NB])
                nc.vector.tensor_add(eam[:], eam[:], wps[:, NB:])

            for b in range(BPC):
                nc.sync.dma_start(out_d[b * NSEQ:(b + 1) * NSEQ, :],
                                  ea[b * 32:b * 32 + NSEQ, :])

    nc.compile()
    return nc


_NC_CACHE = None
_RUN_CACHE = None


def _get_runner():
    """Build (once) a cached jitted dispatch for the compiled Bass module.

    Mirrors what bass_utils.run_bass_kernel_spmd does under axon
    (bass2jax.run_bass_via_pjrt), but keeps the jitted executable across
    calls so repeat dispatches skip re-trace/re-lowering.
    """
    global _NC_CACHE, _RUN_CACHE
    if _RUN_CACHE is not None:
        return _RUN_CACHE
    import jax
    from jax.sharding import Mesh, PartitionSpec
    from jax.experimental.shard_map import shard_map
    from concourse.bass2jax import (
        install_neuronx_cc_hook, _bass_exec_p, partition_id_tensor,
    )

    if _NC_CACHE is None:
        _NC_CACHE = build_bass()
    nc = _NC_CACHE
    install_neuronx_cc_hook()
    partition_name = nc.partition_id_tensor.name if nc.partition_id_tensor else None
    in_names, out_names, out_avals, zero_shapes = [], [], [], []
    for alloc in nc.m.functions[0].allocations:
        if not isinstance(alloc, mybir.MemoryLocationSet):
            continue
        name = alloc.memorylocations[0].name
        if alloc.kind == "ExternalInput":
            if name != partition_name:
                in_names.append(name)
        elif alloc.kind == "ExternalOutput":
            out_names.append(name)
            shape = tuple(alloc.tensor_shape)
            dtype = mybir.dt.np(alloc.dtype)
            out_avals.append(jax.core.ShapedArray(shape, dtype))
            zero_shapes.append((shape, dtype))
    n_params = len(in_names)
    n_outs = len(out_avals)
    all_names = list(in_names) + out_names
    if partition_name is not None:
        all_names.append(partition_name)
    donate = tuple(range(n_params, n_params + n_outs))

    def _body(*args):
        operands = list(args)
        if partition_name is not None:
            operands.append(partition_id_tensor())
        outs = _bass_exec_p.bind(
            *operands, out_avals=tuple(out_avals), in_names=tuple(all_names),
            out_names=tuple(out_names), lowering_input_output_aliases=(),
            sim_require_finite=True, sim_require_nnan=True, nc=nc)
        return tuple(outs)

    devices = jax.devices()[:NCORES]
    mesh = Mesh(np.asarray(devices), ("core",))
    sharded = jax.jit(
        shard_map(_body, mesh=mesh,
                  in_specs=(PartitionSpec("core"),) * (n_params + n_outs),
                  out_specs=(PartitionSpec("core"),) * n_outs,
                  check_rep=False),
        donate_argnums=donate, keep_unused=True)
    _RUN_CACHE = (sharded, in_names, out_names, out_avals, zero_shapes)
    return _RUN_CACHE


def _dispatch(in_maps):
    """One full dispatch: host concat -> H2D -> execute -> D2H."""
    import jax
    sharded, in_names, out_names, out_avals, zero_shapes = _get_runner()
    concat_in = [
        np.concatenate([np.asarray(m[name]) for m in in_maps], axis=0)
        for name in in_names
    ]
    concat_zeros = [
        np.zeros((NCORES * s[0], *s[1:]), dt) for s, dt in zero_shapes
    ]
    outs = sharded(*concat_in, *concat_zeros)
    outs = [np.asarray(o) for o in outs]
    return [
        {name: outs[i].reshape(NCORES, *out_avals[i].shape)[c]
         for i, name in enumerate(out_names)}
        for c in range(NCORES)
    ]


def kernel(traversal_lists, adj_matrices, ent_attn, spo_attn,
           ctx_idx_adjusted, roi_cls, roi_mask, weight_on_children):
    in_maps = []
    for k in range(NCORES):
        s = slice(k * BPC, (k + 1) * BPC)
        in_maps.append(_host_prep(
            np.asarray(traversal_lists[s]), np.asarray(adj_matrices[s]),
            np.asarray(ent_attn[s]), np.asarray(spo_attn[s]),
            np.asarray(ctx_idx_adjusted[s]), np.asarray(roi_cls[s]),
            np.asarray(roi_mask[s]), np.asarray(weight_on_children[s]),
        ))
    res = _dispatch(in_maps)
    out = np.empty((BS, NSEQ, NB), dtype=np.float32)
    for k in range(NCORES):
        out[k * BPC:(k + 1) * BPC] = res[k]["ea_out"].reshape(BPC, NSEQ, NB)
    return out


# revision 13
# speedup vs baseline: 5.2078x; 1.1654x over previous
import sys

sys.path.insert(0, "/opt/trn_rl_repo")

import numpy as np

import concourse.bass as bass
import concourse.tile as tile
from concourse import bacc, mybir
from concourse._compat import get_trn_type

EPS = 1e-6

BS, NSEQ, NB, NC_, ML = 32, 24, 196, 196, 6
BPC = 4            # batches per core
NCORES = 8
P = 112            # partition chunk for (b,i) rows: 4*196=784 = 7*112
NCHUNK = 7
EBLK = 8           # e-rows per scatter block: f = 8*196 = 1568
NEB = 3            # 24 = 3*8
FB = EBLK * NB     # 1568
EM = NSEQ * NB     # 4704
HALF = 98          # m-half for C^T chunks: 196 = 2*98
NKT = NSEQ * 2     # 48 C^T chunks (e, half)
ROWS = BPC * NB    # 784
INVALID = 255      # idx_bnd u8 invalid marker; mapped negative on device


def _host_prep(trav, adj, ent, spo, ctx, roi_cls, roi_mask, w_child):
    """Per-core (4-batch slice) host index/mask prep. Only int-derived
    index/mask/selector tensors and input reshapes/dtype casts — no float
    math on the attention data."""
    import ml_dtypes
    f32, i16, bf16 = np.float32, np.int16, ml_dtypes.bfloat16
    f8 = ml_dtypes.float8_e4m3
    kcls = (roi_cls != -1).astype(f32)                     # [4, 196]

    rows_b = (np.arange(ROWS) // NB).astype(np.int64)
    rows_i = (np.arange(ROWS) % NB).astype(np.int64)
    ctx_rows = ctx[rows_b, rows_i]                         # [784, 196]

    order = np.argsort(ctx_rows, axis=1, kind="stable")
    rank = np.argsort(order, axis=1, kind="stable")
    m_sorted = np.take_along_axis(ctx_rows, order, axis=1)
    first = np.ones_like(m_sorted, dtype=bool)
    first[:, 1:] = m_sorted[:, 1:] != m_sorted[:, :-1]
    last = np.ones_like(m_sorted, dtype=bool)
    last[:, :-1] = m_sorted[:, :-1] != m_sorted[:, 1:]

    segf = np.where(first, 0, 1)                           # [784, 196]
    bnd = np.where(last, m_sorted, INVALID)                # [784, 196]

    def chunks(a):  # [784, F] -> [112, 7*F]
        return np.concatenate([a[c * P:(c + 1) * P] for c in range(NCHUNK)], axis=1)

    u8 = np.uint8
    rank_t = np.ascontiguousarray(chunks(rank).astype(u8))
    bnd_t = np.ascontiguousarray(chunks(bnd).astype(u8))
    segf_t = np.ascontiguousarray(chunks(segf).astype(u8))
    kcls_chunk = np.ascontiguousarray(
        kcls[rows_b, rows_i].reshape(NCHUNK, P).T.astype(f32))   # [112, 7]

    Mt = np.zeros((128, ML * NSEQ), dtype=bf16)
    sel1 = np.zeros((128, ML * BPC), dtype=f32)
    sel2 = np.zeros((BPC, ML * 128), dtype=f32)
    w_rows = np.zeros((BPC, ML * NB), dtype=bf16)
    eps4 = np.zeros((BPC, ML), dtype=f32)
    for t in range(ML):
        for b in range(BPC):
            p_raw = int(trav[b, t])
            p = max(p_raw, 0)
            edges = adj[b, p]
            cm = (edges >= 0) & (p_raw >= 0)
            ec = np.maximum(edges, 0)
            nch = int(cm.sum())
            for j in range(NSEQ):
                if cm[j]:
                    Mt[b * 32 + j, t * NSEQ + int(ec[j])] = 1.0
            sel1[b * 32 + p, t * BPC + b] = 1.0
            if nch > 0 and p_raw >= 0:
                sel2[b, t * 128 + b * 32 + p] = 1.0
            w_rows[b, t * NB:(t + 1) * NB] = w_child[b, p].astype(bf16)
            eps4[b, t] = max(nch, 1) * EPS

    ea0 = np.ascontiguousarray(ent.reshape(BPC * NSEQ, NB).astype(f32))
    kclsr = np.zeros((128, NB), dtype=bf16)
    for b in range(BPC):
        kclsr[b * 32:b * 32 + NSEQ] = kcls[b][None, :].astype(bf16)

    return {
        "spo": np.ascontiguousarray(spo.transpose(0, 2, 1, 3)).astype(f8),
        "roi": roi_mask.astype(u8),
        "rank_c": rank_t,
        "bnd_c": bnd_t,
        "segf_c": segf_t,
        "kcls_chunk": kcls_chunk,
        "Mt": Mt, "sel1": sel1, "sel2": sel2,
        "w_rows": w_rows, "eps4": eps4,
        "ea0": ea0, "kclsr": kclsr,
        "kcls4": kcls.astype(f32),
        "ident": np.eye(P, dtype=bf16),
    }


def _row_ranges(c):
    """(b, i0, i1, q0) sub-ranges of chunk c at batch boundaries."""
    r0, r1 = c * P, (c + 1) * P
    out = []
    r = r0
    while r < r1:
        b = r // NB
        i0 = r % NB
        i1 = min(NB, i0 + (r1 - r))
        out.append((b, i0, i1, r - r0))
        r += i1 - i0
    return out


def build_bass():
    f32 = mybir.dt.float32
    bf16 = mybir.dt.bfloat16
    i16 = mybir.dt.int16
    f8 = mybir.dt.float8e4
    u8 = mybir.dt.uint8
    nc = bacc.Bacc(get_trn_type() or "TRN2", target_bir_lowering=False)

    spo_d = nc.dram_tensor("spo", (BPC, NB, NSEQ, NC_), f8, kind="ExternalInput")
    roi_d = nc.dram_tensor("roi", (BPC, NB, NC_), u8, kind="ExternalInput")
    rank_d = nc.dram_tensor("rank_c", (P, NCHUNK * NC_), u8, kind="ExternalInput")
    bnd_d = nc.dram_tensor("bnd_c", (P, NCHUNK * NC_), u8, kind="ExternalInput")
    segf_d = nc.dram_tensor("segf_c", (P, NCHUNK * NC_), u8, kind="ExternalInput")
    kch_d = nc.dram_tensor("kcls_chunk", (P, NCHUNK), f32, kind="ExternalInput")
    Mt_d = nc.dram_tensor("Mt", (128, ML * NSEQ), bf16, kind="ExternalInput")
    sel1_d = nc.dram_tensor("sel1", (128, ML * BPC), f32, kind="ExternalInput")
    sel2_d = nc.dram_tensor("sel2", (BPC, ML * 128), f32, kind="ExternalInput")
    wr_d = nc.dram_tensor("w_rows", (BPC, ML * NB), bf16, kind="ExternalInput")
    eps_d = nc.dram_tensor("eps4", (BPC, ML), f32, kind="ExternalInput")
    ea0_d = nc.dram_tensor("ea0", (BPC * NSEQ, NB), f32, kind="ExternalInput")
    kclsr_d = nc.dram_tensor("kclsr", (128, NB), bf16, kind="ExternalInput")
    kcls4_d = nc.dram_tensor("kcls4", (BPC, NB), f32, kind="ExternalInput")
    id_d = nc.dram_tensor("ident", (P, P), bf16, kind="ExternalInput")
    out_d = nc.dram_tensor("ea_out", (BPC * NSEQ, NB), f32, kind="ExternalOutput")

    with tile.TileContext(nc) as tc:
        with (
            tc.tile_pool(name="persist", bufs=1) as pp,
            tc.tile_pool(name="stage", bufs=2) as sp,
            tc.tile_pool(name="work", bufs=2) as wp,
            tc.tile_pool(name="small", bufs=2) as mp,
            tc.tile_pool(name="psA", bufs=2, space="PSUM") as psA,
            tc.tile_pool(name="psB", bufs=1, space="PSUM") as psB,
        ):
            # ---- persistent tiles ----
            CT = pp.tile([HALF, NKT * ROWS], bf16, tag="CT")
            ea = pp.tile([128, NB], f32, tag="ea")
            eam = pp.tile([128, NB], bf16, tag="eam")
            kch = pp.tile([P, NCHUNK], f32, tag="kch")
            Mt = pp.tile([128, ML * NSEQ], bf16, tag="Mt")
            sel1 = pp.tile([128, ML * BPC], f32, tag="sel1")
            sel2 = pp.tile([BPC, ML * 128], f32, tag="sel2")
            wr = pp.tile([BPC, ML * NB], bf16, tag="wr")
            eps4 = pp.tile([BPC, ML], f32, tag="eps4")
            kclsr = pp.tile([128, NB], bf16, tag="kclsr")
            kcls4 = pp.tile([BPC, NB], f32, tag="kcls4")
            mm1 = pp.tile([BPC, NB], f32, tag="mm1")
            ident = pp.tile([P, P], bf16, tag="ident")
            ones4 = pp.tile([HALF, BPC], f32, tag="ones4")
            acc = pp.tile([HALF, ROWS], f32, tag="acc")

            for dst, src in [
                (kch, kch_d), (Mt, Mt_d), (sel1, sel1_d), (sel2, sel2_d),
                (wr, wr_d), (eps4, eps_d), (kclsr, kclsr_d),
                (kcls4, kcls4_d), (ident, id_d),
            ]:
                nc.sync.dma_start(dst[:], src[:])
            nc.vector.memset(ea[:], 0.0)
            for b in range(BPC):
                nc.sync.dma_start(ea[b * 32:b * 32 + NSEQ, :],
                                  ea0_d[b * NSEQ:(b + 1) * NSEQ, :])
            nc.vector.tensor_mul(eam[:], ea[:], kclsr[:])
            nc.vector.tensor_scalar_add(mm1[:], kcls4[:], -1.0)
            nc.vector.memset(ones4[:], 1.0)

            # ---- per chunk: spo3 -> scatter -> scan -> extract -> transpose ----
            for c in range(NCHUNK):
                st = sp.tile([P, NSEQ, NC_], f8, tag="spost")
                for (b, i0, i1, q0) in _row_ranges(c):
                    nc.sync.dma_start(
                        st[q0:q0 + (i1 - i0), :, :],
                        spo_d[b, i0:i1, :, :],
                    )
                rt = sp.tile([P, NC_], u8, tag="roist")
                for (b, i0, i1, q0) in _row_ranges(c):
                    nc.sync.dma_start(rt[q0:q0 + (i1 - i0), :], roi_d[b, i0:i1, :])
                # roi is binary so roi^3 * kcls == roi * kcls
                w3c = wp.tile([P, NC_], f32, tag="w3c")
                nc.vector.tensor_scalar_mul(w3c[:], rt[:], kch[:, c:c + 1])
                sp3c = wp.tile([P, EM], bf16, tag="sp3c")
                w3b = w3c[:].unsqueeze(1).broadcast_to((P, NSEQ, NC_))
                nc.vector.tensor_mul(sp3c[:].rearrange("p (e c) -> p e c", e=NSEQ),
                                     st[:], w3b)
                # expand compact per-row idx/segment tensors to 8 e-rows
                rankc = wp.tile([P, NC_], u8, tag="rankc")
                bnd0 = wp.tile([P, NC_], u8, tag="bnd0")
                seg0 = wp.tile([P, NC_], u8, tag="seg0")
                nc.sync.dma_start(rankc[:], rank_d[:, c * NC_:(c + 1) * NC_])
                nc.sync.dma_start(bnd0[:], bnd_d[:, c * NC_:(c + 1) * NC_])
                nc.sync.dma_start(seg0[:], segf_d[:, c * NC_:(c + 1) * NC_])
                # bnd: 255 marks invalid; map to negative so +e*196 stays < 0
                bfix = wp.tile([P, NC_], i16, tag="bfix")
                nc.vector.tensor_scalar(bfix[:], bnd0[:], 255, -9816,
                                        op0=mybir.AluOpType.is_equal,
                                        op1=mybir.AluOpType.mult)
                nc.vector.tensor_tensor(bfix[:], bfix[:], bnd0[:],
                                        op=mybir.AluOpType.add)
                sigc = wp.tile([P, FB], i16, tag="sigc")
                bndc = wp.tile([P, FB], i16, tag="bndc")
                segc = wp.tile([P, FB], bf16, tag="segc")
                for e in range(EBLK):
                    s = slice(e * NC_, (e + 1) * NC_)
                    nc.vector.tensor_scalar_add(sigc[:, s], rankc[:], e * NC_)
                    nc.vector.tensor_scalar_add(bndc[:, s], bfix[:], e * NC_)
                    nc.scalar.copy(segc[:, s], seg0[:])
                Cmc = wp.tile([P, EM], bf16, tag="Cmc")
                for e in range(NEB):
                    fb0 = e * FB
                    srt = wp.tile([P, FB], bf16, tag="sorted")
                    nc.gpsimd.local_scatter(
                        srt[:], sp3c[:, fb0:fb0 + FB], sigc[:],
                        channels=P, num_elems=FB, num_idxs=FB,
                    )
                    scn = wp.tile([P, FB], bf16, tag="scan")
                    nc.vector.tensor_tensor_scan(
                        scn[:], segc[:], srt[:], 0.0,
                        op0=mybir.AluOpType.mult, op1=mybir.AluOpType.add,
                    )
                    nc.gpsimd.local_scatter(
                        Cmc[:, fb0:fb0 + FB], scn[:], bndc[:],
                        channels=P, num_elems=FB, num_idxs=FB,
                    )
                for g in range(NKT // 4):
                    pt4 = psA.tile([HALF, 4, P], bf16, tag="tp")
                    for j in range(4):
                        s = g * 4 + j
                        nc.tensor.transpose(
                            pt4[:, j, :], Cmc[:, s * HALF:(s + 1) * HALF],
                            ident[:])
                    dst = (CT[:, 4 * g * ROWS: 4 * (g + 1) * ROWS]
                           .rearrange("p (s r) -> p s r", s=4)
                           [:, :, c * P:(c + 1) * P])
                    nc.scalar.copy(dst, pt4[:])

            # ---- 6 sequential steps ----
            for t in range(ML):
                a4 = [mp.tile([HALF, NSEQ, BPC], bf16, tag=f"a4_{h}",
                              name=f"a4_{h}") for h in range(2)]
                for h in range(2):
                    for b in range(BPC):
                        aps = psA.tile([HALF, NSEQ], f32, tag="aps")
                        nc.tensor.matmul(
                            aps[:],
                            eam[b * 32:b * 32 + NSEQ, h * HALF:(h + 1) * HALF],
                            Mt[b * 32:b * 32 + NSEQ, t * NSEQ:(t + 1) * NSEQ],
                            start=True, stop=True,
                            tile_position=(b * 32, 0),
                        )
                        nc.scalar.copy(a4[h][:, :, b], aps[:])
                KPE = 34
                rps = [psB.tile([BPC, 2 * NB], f32, tag=f"rps{nb}",
                                name=f"rps{nb}") for nb in range(2)]
                for k in range(NKT):
                    e, h = k // 2, k % 2
                    if k < KPE:
                        for nb in range(2):
                            nc.tensor.matmul(
                                rps[nb][:],
                                a4[h][:, e, :],
                                CT[:, k * ROWS + nb * 2 * NB: k * ROWS + (nb + 1) * 2 * NB],
                                start=(k == 0), stop=False,
                            )
                    else:
                        for b in range(BPC):
                            nc.vector.scalar_tensor_tensor(
                                acc[:, b * NB:(b + 1) * NB],
                                CT[:, k * ROWS + b * NB: k * ROWS + (b + 1) * NB],
                                a4[h][:, e, b:b + 1],
                                acc[:, b * NB:(b + 1) * NB],
                                op0=mybir.AluOpType.mult,
                                op1=(mybir.AluOpType.add if k > KPE
                                     else mybir.AluOpType.bypass),
                            )
                for nb in range(2):
                    nc.tensor.matmul(
                        rps[nb][:], ones4[:],
                        acc[:, nb * 2 * NB:(nb + 1) * 2 * NB],
                        start=False, stop=(nb == 1),
                    )
                r4 = mp.tile([BPC, NB], f32, tag="r4")
                for nb in range(2):
                    rsb = mp.tile([BPC, 2 * NB], f32, tag=f"rsb{nb}",
                                  name=f"rsb{nb}", bufs=1)
                    nc.vector.tensor_copy(rsb[:], rps[nb][:])
                    for b in (2 * nb, 2 * nb + 1):
                        nc.sync.dma_start(
                            r4[b:b + 1, :],
                            rsb[b:b + 1, (b % 2) * NB:(b % 2) * NB + NB])
                nc.vector.tensor_scalar_add(r4[:], r4[:], eps4[:, t:t + 1])
                sps = psB.tile([BPC, NB], f32, tag="sps")
                nc.tensor.matmul(sps[:], sel1[:, t * BPC:(t + 1) * BPC], ea[:],
                                 start=True, stop=True)
                srow = mp.tile([BPC, NB], f32, tag="srow")
                nc.vector.tensor_copy(srow[:], sps[:])
                upd = mp.tile([BPC, NB], f32, tag="upd")
                nc.vector.tensor_mul(upd[:], r4[:], wr[:, t * NB:(t + 1) * NB])
                nc.vector.tensor_add(upd[:], upd[:], srow[:])
                nrm = mp.tile([BPC, 1], f32, tag="nrm")
                nc.vector.tensor_reduce(nrm[:], upd[:], axis=mybir.AxisListType.X,
                                        op=mybir.AluOpType.max,
                                        apply_absolute_value=True)
                nc.vector.tensor_scalar_max(nrm[:], nrm[:], 1.0)
                rec = mp.tile([BPC, 1], f32, tag="rec")
                nc.vector.reciprocal(rec[:], nrm[:])
                nc.vector.tensor_scalar_mul(upd[:], upd[:], rec[:])
                nc.vector.tensor_mul(upd[:], upd[:], kcls4[:])
                nc.vector.tensor_add(upd[:], upd[:], mm1[:])
                dd = mp.tile([BPC, 2 * NB], f32, tag="dd", bufs=1)
                nc.vector.tensor_sub(dd[:, :NB], upd[:], srow[:])
                nc.vector.tensor_mul(dd[:, NB:], dd[:, :NB], kcls4[:])
                wps = psB.tile([128, 2 * NB], f32, tag="wps")
                nc.tensor.matmul(wps[:], sel2[:, t * 128:(t + 1) * 128], dd[:],
                                 start=True, stop=True)
                nc.vector.tensor_add(ea[:], ea[:], wps[:, :NB])
                nc.vector.tensor_add(eam[:], eam[:], wps[:, NB:])

            for b in range(BPC):
                nc.sync.dma_start(out_d[b * NSEQ:(b + 1) * NSEQ, :],
                                  ea[b * 32:b * 32 + NSEQ, :])

    nc.compile()
    return nc


_NC_CACHE = None
_RUN_CACHE = None


def _get_runner():
    """Build (once) a cached jitted dispatch for the compiled Bass module.

    Mirrors what bass_utils.run_bass_kernel_spmd does under axon
    (bass2jax.run_bass_via_pjrt), but keeps the jitted executable across
    calls so repeat dispatches skip re-trace/re-lowering.
    """
    global _NC_CACHE, _RUN_CACHE
    if _RUN_CACHE is not None:
        return _RUN_CACHE
    import jax
    from jax.sharding import Mesh, PartitionSpec
    from jax.experimental.shard_map import shard_map
    from concourse.bass2jax import (
        install_neuronx_cc_hook, _bass_exec_p, partition_id_tensor,
    )

    if _NC_CACHE is None:
        _NC_CACHE = build_bass()
    nc = _NC_CACHE
    install_neuronx_cc_hook()
    partition_name = nc.partition_id_tensor.name if nc.partition_id_tensor else None
    in_names, out_names, out_avals, zero_shapes = [], [], [], []
    for alloc in nc.m.functions[0].allocations:
        if not isinstance(alloc, mybir.MemoryLocationSet):
            continue
        name = alloc.memorylocations[0].name
        if alloc.kind == "ExternalInput":
            if name != partition_name:
                in_names.append(name)
        elif alloc.kind == "ExternalOutput":
            out_names.append(name)
            shape = tuple(alloc.tensor_shape)
            dtype = mybir.dt.np(alloc.dtype)
            out_avals.append(jax.core.ShapedArray(shape, dtype))
            zero_shapes.append((shape, dtype))
    n_params = len(in_names)
    n_outs = len(out_avals)
    all_names = list(in_names) + out_names
    if partition_name is not None:
        all_names.append(partition_name)
    donate = tuple(range(n_params, n_params + n_outs))

    def _body(*args):
        operands = list(args)
        if partition_name is not None:
            operands.append(partition_id_tensor())
        outs = _bass_exec_p.bind(
            *operands, out_avals=tuple(out_avals), in_names=tuple(all_names),
            out_names=tuple(out_names), lowering_input_output_aliases=(),
            sim_require_finite=True, sim_require_nnan=True, nc=nc)
        return tuple(outs)

    devices = jax.devices()[:NCORES]
    mesh = Mesh(np.asarray(devices), ("core",))
    sharded = jax.jit(
        shard_map(_body, mesh=mesh,
                  in_specs=(PartitionSpec("core"),) * (n_params + n_outs),
                  out_specs=(PartitionSpec("core"),) * n_outs,
                  check_rep=False),
        donate_argnums=donate, keep_unused=True)
    _RUN_CACHE = (sharded, in_names, out_names, out_avals, zero_shapes)
    return _RUN_CACHE


def _dispatch(in_maps):
    """One full dispatch: host concat -> H2D -> execute -> D2H."""
    import jax
    sharded, in_names, out_names, out_avals, zero_shapes = _get_runner()
    concat_in = [
        np.concatenate([np.asarray(m[name]) for m in in_maps], axis=0)
        for name in in_names
    ]
    concat_zeros = [
        np.zeros((NCORES * s[0], *s[1:]), dt) for s, dt in zero_shapes
    ]
    outs = sharded(*concat_in, *concat_zeros)
    outs = [np.asarray(o) for o in outs]
    return [
        {name: outs[i].reshape(NCORES, *out_avals[i].shape)[c]
         for i, name in enumerate(out_names)}
        for c in range(NCORES)
    ]


def kernel(traversal_lists, adj_matrices, ent_attn, spo_attn,
           ctx_idx_adjusted, roi_cls, roi_mask, weight_on_children):
    in_maps = []
    for k in range(NCORES):
        s = slice(k * BPC, (k + 1) * BPC)
        in_maps.append(_host_prep(
            np.asarray(traversal_lists[s]), np.asarray(adj_matrices[s]),
            np.asarray(ent_attn[s]), np.asarray(spo_attn[s]),
            np.asarray(ctx_idx_adjusted[s]), np.asarray(roi_cls[s]),
            np.asarray(roi_mask[s]), np.asarray(weight_on_children[s]),
        ))
    res = _dispatch(in_maps)
    out = np.empty((BS, NSEQ, NB), dtype=np.float32)
    for k in range(NCORES):
        out[k * BPC:(k + 1) * BPC] = res[k]["ea_out"].reshape(BPC, NSEQ, NB)
    return out


# revision 14
# speedup vs baseline: 5.6706x; 1.0889x over previous
import sys

sys.path.insert(0, "/opt/trn_rl_repo")

import numpy as np

import concourse.bass as bass
import concourse.tile as tile
from concourse import bacc, mybir
from concourse._compat import get_trn_type

EPS = 1e-6

BS, NSEQ, NB, NC_, ML = 32, 24, 196, 196, 6
BPC = 4            # batches per core
NCORES = 8
P = 112            # partition chunk for (b,i) rows: 4*196=784 = 7*112
NCHUNK = 7
EBLK = 8           # e-rows per scatter block: f = 8*196 = 1568
NEB = 3            # 24 = 3*8
FB = EBLK * NB     # 1568
EM = NSEQ * NB     # 4704
HALF = 98          # m-half for C^T chunks: 196 = 2*98
NKT = NSEQ * 2     # 48 C^T chunks (e, half)
ROWS = BPC * NB    # 784
INVALID = 255      # idx_bnd u8 invalid marker; mapped negative on device

# ---- packed single-input layout (per core), byte offsets ----
# f32 section first so every offset stays 4-aligned, then bf16, then 1-byte.
_PACK_SPEC = [
    ("kch",   "f32",  (P, NCHUNK)),
    ("sel1",  "f32",  (128, ML * BPC)),
    ("sel2",  "f32",  (BPC, ML * 128)),
    ("eps4",  "f32",  (BPC, ML)),
    ("ea0",   "f32",  (BPC * NSEQ, NB)),
    ("kcls4", "f32",  (BPC, NB)),
    ("Mt",    "bf16", (128, ML * NSEQ)),
    ("wr",    "bf16", (BPC, ML * NB)),
    ("kclsr", "bf16", (128, NB)),
    ("ident", "bf16", (P, P)),
    ("spo",   "f8",   (ROWS, NSEQ, NC_)),
    ("roi",   "u8",   (ROWS, NC_)),
    ("rank",  "u8",   (NCHUNK, P, NC_)),
    ("bnd",   "u8",   (NCHUNK, P, NC_)),
    ("segf",  "u8",   (NCHUNK, P, NC_)),
]
_DTSIZE = {"f32": 4, "bf16": 2, "f8": 1, "u8": 1}
PACK_OFF = {}
_off = 0
for _n, _dt, _shape in _PACK_SPEC:
    PACK_OFF[_n] = _off
    _sz = _DTSIZE[_dt]
    for _d in _shape:
        _sz *= _d
    _off += _sz
PACK_BYTES = _off


def _host_prep(trav, adj, ent, spo, ctx, roi_cls, roi_mask, w_child):
    """Per-core (4-batch slice) host index/mask prep. Only int-derived
    index/mask/selector tensors and input reshapes/dtype casts — no float
    math on the attention data. Returns one packed u8 blob."""
    import ml_dtypes
    f32, u8, bf16 = np.float32, np.uint8, ml_dtypes.bfloat16
    f8 = ml_dtypes.float8_e4m3
    kcls = (roi_cls != -1).astype(f32)                     # [4, 196]

    rows_b = (np.arange(ROWS) // NB).astype(np.int64)
    rows_i = (np.arange(ROWS) % NB).astype(np.int64)
    ctx_rows = ctx[rows_b, rows_i]                         # [784, 196]

    order = np.argsort(ctx_rows, axis=1, kind="stable")
    rank = np.argsort(order, axis=1, kind="stable")
    m_sorted = np.take_along_axis(ctx_rows, order, axis=1)
    first = np.ones_like(m_sorted, dtype=bool)
    first[:, 1:] = m_sorted[:, 1:] != m_sorted[:, :-1]
    last = np.ones_like(m_sorted, dtype=bool)
    last[:, :-1] = m_sorted[:, :-1] != m_sorted[:, 1:]

    segf = np.where(first, 0, 1)                           # [784, 196]
    bnd = np.where(last, m_sorted, INVALID)                # [784, 196]

    def chunks(a):  # [784, F] -> [7, 112, F]
        return np.ascontiguousarray(a.reshape(NCHUNK, P, -1))

    kch = np.ascontiguousarray(
        kcls[rows_b, rows_i].reshape(NCHUNK, P).T.astype(f32))   # [112, 7]

    Mt = np.zeros((128, ML * NSEQ), dtype=bf16)
    sel1 = np.zeros((128, ML * BPC), dtype=f32)
    sel2 = np.zeros((BPC, ML * 128), dtype=f32)
    w_rows = np.zeros((BPC, ML * NB), dtype=bf16)
    eps4 = np.zeros((BPC, ML), dtype=f32)
    for t in range(ML):
        for b in range(BPC):
            p_raw = int(trav[b, t])
            p = max(p_raw, 0)
            edges = adj[b, p]
            cm = (edges >= 0) & (p_raw >= 0)
            ec = np.maximum(edges, 0)
            nch = int(cm.sum())
            for j in range(NSEQ):
                if cm[j]:
                    Mt[b * 32 + j, t * NSEQ + int(ec[j])] = 1.0
            sel1[b * 32 + p, t * BPC + b] = 1.0
            if nch > 0 and p_raw >= 0:
                sel2[b, t * 128 + b * 32 + p] = 1.0
            w_rows[b, t * NB:(t + 1) * NB] = w_child[b, p].astype(bf16)
            eps4[b, t] = max(nch, 1) * EPS

    ea0 = np.ascontiguousarray(ent.reshape(BPC * NSEQ, NB).astype(f32))
    kclsr = np.zeros((128, NB), dtype=bf16)
    for b in range(BPC):
        kclsr[b * 32:b * 32 + NSEQ] = kcls[b][None, :].astype(bf16)

    sections = {
        "kch": kch,
        "sel1": sel1, "sel2": sel2, "eps4": eps4,
        "ea0": ea0, "kcls4": kcls.astype(f32),
        "Mt": Mt, "wr": w_rows, "kclsr": kclsr,
        "ident": np.eye(P, dtype=bf16),
        "spo": np.ascontiguousarray(
            spo.transpose(0, 2, 1, 3)).astype(f8).reshape(ROWS, NSEQ, NC_),
        "roi": roi_mask.astype(u8).reshape(ROWS, NC_),
        "rank": chunks(rank.astype(u8)),
        "bnd": chunks(bnd.astype(u8)),
        "segf": chunks(segf.astype(u8)),
    }
    pack = np.concatenate(
        [np.ascontiguousarray(sections[n]).reshape(-1).view(np.uint8)
         for n, _, _ in _PACK_SPEC])
    assert pack.nbytes == PACK_BYTES
    return {"pack": pack}


def build_bass():
    f32 = mybir.dt.float32
    bf16 = mybir.dt.bfloat16
    i16 = mybir.dt.int16
    f8 = mybir.dt.float8e4
    u8 = mybir.dt.uint8
    DT = {"f32": f32, "bf16": bf16, "f8": f8, "u8": u8}
    nc = bacc.Bacc(get_trn_type() or "TRN2", target_bir_lowering=False)

    pack_d = nc.dram_tensor("pack", (PACK_BYTES,), u8, kind="ExternalInput")
    out_d = nc.dram_tensor("ea_out", (BPC * NSEQ, NB), f32, kind="ExternalOutput")

    spec = {n: (dt, shape) for n, dt, shape in _PACK_SPEC}

    def view(name, sub_off=0, shape=None):
        dt, full_shape = spec[name]
        shape = shape if shape is not None else full_shape
        n = 1
        for d in shape:
            n *= d
        sz = _DTSIZE[dt]
        ap = pack_d[PACK_OFF[name] + sub_off * sz:
                    PACK_OFF[name] + (sub_off + n) * sz].bitcast(DT[dt])
        if len(shape) == 2:
            ap = ap.rearrange("(a b) -> a b", a=shape[0])
        elif len(shape) == 3:
            ap = ap.rearrange("(a b c) -> a b c", a=shape[0], b=shape[1])
        return ap

    with tile.TileContext(nc) as tc:
        with (
            tc.tile_pool(name="persist", bufs=1) as pp,
            tc.tile_pool(name="stage", bufs=2) as sp,
            tc.tile_pool(name="work", bufs=2) as wp,
            tc.tile_pool(name="small", bufs=2) as mp,
            tc.tile_pool(name="psA", bufs=2, space="PSUM") as psA,
            tc.tile_pool(name="psB", bufs=1, space="PSUM") as psB,
        ):
            # ---- persistent tiles ----
            CT = pp.tile([HALF, NKT * ROWS], bf16, tag="CT")
            ea = pp.tile([128, NB], f32, tag="ea")
            eam = pp.tile([128, NB], bf16, tag="eam")
            kch = pp.tile([P, NCHUNK], f32, tag="kch")
            Mt = pp.tile([128, ML * NSEQ], bf16, tag="Mt")
            sel1 = pp.tile([128, ML * BPC], f32, tag="sel1")
            sel2 = pp.tile([BPC, ML * 128], f32, tag="sel2")
            wr = pp.tile([BPC, ML * NB], bf16, tag="wr")
            eps4 = pp.tile([BPC, ML], f32, tag="eps4")
            kclsr = pp.tile([128, NB], bf16, tag="kclsr")
            kcls4 = pp.tile([BPC, NB], f32, tag="kcls4")
            mm1 = pp.tile([BPC, NB], f32, tag="mm1")
            ident = pp.tile([P, P], bf16, tag="ident")
            ones4 = pp.tile([HALF, BPC], f32, tag="ones4")
            acc = pp.tile([HALF, ROWS], f32, tag="acc")

            for dst, name in [
                (kch, "kch"), (Mt, "Mt"), (sel1, "sel1"), (sel2, "sel2"),
                (wr, "wr"), (eps4, "eps4"), (kclsr, "kclsr"),
                (kcls4, "kcls4"), (ident, "ident"),
            ]:
                nc.sync.dma_start(dst[:], view(name))
            nc.vector.memset(ea[:], 0.0)
            for b in range(BPC):
                nc.sync.dma_start(
                    ea[b * 32:b * 32 + NSEQ, :],
                    view("ea0", sub_off=b * NSEQ * NB, shape=(NSEQ, NB)))
            nc.vector.tensor_mul(eam[:], ea[:], kclsr[:])
            nc.vector.tensor_scalar_add(mm1[:], kcls4[:], -1.0)
            nc.vector.memset(ones4[:], 1.0)

            # ---- per chunk: spo3 -> scatter -> scan -> extract -> transpose ----
            for c in range(NCHUNK):
                st = sp.tile([P, NSEQ, NC_], f8, tag="spost")
                nc.sync.dma_start(
                    st[:], view("spo", sub_off=c * P * NSEQ * NC_,
                                shape=(P, NSEQ, NC_)))
                rt = sp.tile([P, NC_], u8, tag="roist")
                nc.sync.dma_start(
                    rt[:], view("roi", sub_off=c * P * NC_, shape=(P, NC_)))
                # roi is binary so roi^3 * kcls == roi * kcls
                w3c = wp.tile([P, NC_], f32, tag="w3c")
                nc.vector.tensor_scalar_mul(w3c[:], rt[:], kch[:, c:c + 1])
                sp3c = wp.tile([P, EM], bf16, tag="sp3c")
                w3b = w3c[:].unsqueeze(1).broadcast_to((P, NSEQ, NC_))
                nc.vector.tensor_mul(sp3c[:].rearrange("p (e c) -> p e c", e=NSEQ),
                                     st[:], w3b)
                # expand compact per-row idx/segment tensors to 8 e-rows
                rankc = wp.tile([P, NC_], u8, tag="rankc")
                bnd0 = wp.tile([P, NC_], u8, tag="bnd0")
                seg0 = wp.tile([P, NC_], u8, tag="seg0")
                nc.sync.dma_start(
                    rankc[:], view("rank", sub_off=c * P * NC_, shape=(P, NC_)))
                nc.sync.dma_start(
                    bnd0[:], view("bnd", sub_off=c * P * NC_, shape=(P, NC_)))
                nc.sync.dma_start(
                    seg0[:], view("segf", sub_off=c * P * NC_, shape=(P, NC_)))
                # bnd: 255 marks invalid; map to negative so +e*196 stays < 0
                bfix = wp.tile([P, NC_], i16, tag="bfix")
                nc.vector.tensor_scalar(bfix[:], bnd0[:], 255, -9816,
                                        op0=mybir.AluOpType.is_equal,
                                        op1=mybir.AluOpType.mult)
                nc.vector.tensor_tensor(bfix[:], bfix[:], bnd0[:],
                                        op=mybir.AluOpType.add)
                sigc = wp.tile([P, FB], i16, tag="sigc")
                bndc = wp.tile([P, FB], i16, tag="bndc")
                segc = wp.tile([P, FB], bf16, tag="segc")
                for e in range(EBLK):
                    s = slice(e * NC_, (e + 1) * NC_)
                    nc.vector.tensor_scalar_add(sigc[:, s], rankc[:], e * NC_)
                    nc.vector.tensor_scalar_add(bndc[:, s], bfix[:], e * NC_)
                    nc.scalar.copy(segc[:, s], seg0[:])
                Cmc = wp.tile([P, EM], bf16, tag="Cmc")
                for e in range(NEB):
                    fb0 = e * FB
                    srt = wp.tile([P, FB], bf16, tag="sorted")
                    nc.gpsimd.local_scatter(
                        srt[:], sp3c[:, fb0:fb0 + FB], sigc[:],
                        channels=P, num_elems=FB, num_idxs=FB,
                    )
                    scn = wp.tile([P, FB], bf16, tag="scan")
                    nc.vector.tensor_tensor_scan(
                        scn[:], segc[:], srt[:], 0.0,
                        op0=mybir.AluOpType.mult, op1=mybir.AluOpType.add,
                    )
                    nc.gpsimd.local_scatter(
                        Cmc[:, fb0:fb0 + FB], scn[:], bndc[:],
                        channels=P, num_elems=FB, num_idxs=FB,
                    )
                for g in range(NKT // 4):
                    pt4 = psA.tile([HALF, 4, P], bf16, tag="tp")
                    for j in range(4):
                        s = g * 4 + j
                        nc.tensor.transpose(
                            pt4[:, j, :], Cmc[:, s * HALF:(s + 1) * HALF],
                            ident[:])
                    dst = (CT[:, 4 * g * ROWS: 4 * (g + 1) * ROWS]
                           .rearrange("p (s r) -> p s r", s=4)
                           [:, :, c * P:(c + 1) * P])
                    nc.scalar.copy(dst, pt4[:])

            # ---- 6 sequential steps ----
            for t in range(ML):
                a4 = [mp.tile([HALF, NSEQ, BPC], bf16, tag=f"a4_{h}",
                              name=f"a4_{h}") for h in range(2)]
                for h in range(2):
                    for b in range(BPC):
                        aps = psA.tile([HALF, NSEQ], f32, tag="aps")
                        nc.tensor.matmul(
                            aps[:],
                            eam[b * 32:b * 32 + NSEQ, h * HALF:(h + 1) * HALF],
                            Mt[b * 32:b * 32 + NSEQ, t * NSEQ:(t + 1) * NSEQ],
                            start=True, stop=True,
                            tile_position=(b * 32, 0),
                        )
                        nc.scalar.copy(a4[h][:, :, b], aps[:])
                KPE = 34
                rps = [psB.tile([BPC, 2 * NB], f32, tag=f"rps{nb}",
                                name=f"rps{nb}") for nb in range(2)]
                for k in range(NKT):
                    e, h = k // 2, k % 2
                    if k < KPE:
                        for nb in range(2):
                            nc.tensor.matmul(
                                rps[nb][:],
                                a4[h][:, e, :],
                                CT[:, k * ROWS + nb * 2 * NB: k * ROWS + (nb + 1) * 2 * NB],
                                start=(k == 0), stop=False,
                            )
                    else:
                        for b in range(BPC):
                            nc.vector.scalar_tensor_tensor(
                                acc[:, b * NB:(b + 1) * NB],
                                CT[:, k * ROWS + b * NB: k * ROWS + (b + 1) * NB],
                                a4[h][:, e, b:b + 1],
                                acc[:, b * NB:(b + 1) * NB],
                                op0=mybir.AluOpType.mult,
                                op1=(mybir.AluOpType.add if k > KPE
                                     else mybir.AluOpType.bypass),
                            )
                for nb in range(2):
                    nc.tensor.matmul(
                        rps[nb][:], ones4[:],
                        acc[:, nb * 2 * NB:(nb + 1) * 2 * NB],
                        start=False, stop=(nb == 1),
                    )
                r4 = mp.tile([BPC, NB], f32, tag="r4")
                for nb in range(2):
                    rsb = mp.tile([BPC, 2 * NB], f32, tag=f"rsb{nb}",
                                  name=f"rsb{nb}", bufs=1)
                    nc.vector.tensor_copy(rsb[:], rps[nb][:])
                    for b in (2 * nb, 2 * nb + 1):
                        nc.sync.dma_start(
                            r4[b:b + 1, :],
                            rsb[b:b + 1, (b % 2) * NB:(b % 2) * NB + NB])
                nc.vector.tensor_scalar_add(r4[:], r4[:], eps4[:, t:t + 1])
                sps = psB.tile([BPC, NB], f32, tag="sps")
                nc.tensor.matmul(sps[:], sel1[:, t * BPC:(t + 1) * BPC], ea[:],
                                 start=True, stop=True)
                srow = mp.tile([BPC, NB], f32, tag="srow")
                nc.vector.tensor_copy(srow[:], sps[:])
                upd = mp.tile([BPC, NB], f32, tag="upd")
                nc.vector.tensor_mul(upd[:], r4[:], wr[:, t * NB:(t + 1) * NB])
                nc.vector.tensor_add(upd[:], upd[:], srow[:])
                nrm = mp.tile([BPC, 1], f32, tag="nrm")
                nc.vector.tensor_reduce(nrm[:], upd[:], axis=mybir.AxisListType.X,
                                        op=mybir.AluOpType.max,
                                        apply_absolute_value=True)
                nc.vector.tensor_scalar_max(nrm[:], nrm[:], 1.0)
                rec = mp.tile([BPC, 1], f32, tag="rec")
                nc.vector.reciprocal(rec[:], nrm[:])
                nc.vector.tensor_scalar_mul(upd[:], upd[:], rec[:])
                nc.vector.tensor_mul(upd[:], upd[:], kcls4[:])
                nc.vector.tensor_add(upd[:], upd[:], mm1[:])
                dd = mp.tile([BPC, 2 * NB], f32, tag="dd", bufs=1)
                nc.vector.tensor_sub(dd[:, :NB], upd[:], srow[:])
                nc.vector.tensor_mul(dd[:, NB:], dd[:, :NB], kcls4[:])
                wps = psB.tile([128, 2 * NB], f32, tag="wps")
                nc.tensor.matmul(wps[:], sel2[:, t * 128:(t + 1) * 128], dd[:],
                                 start=True, stop=True)
                nc.vector.tensor_add(ea[:], ea[:], wps[:, :NB])
                nc.vector.tensor_add(eam[:], eam[:], wps[:, NB:])

            for b in range(BPC):
                nc.sync.dma_start(out_d[b * NSEQ:(b + 1) * NSEQ, :],
                                  ea[b * 32:b * 32 + NSEQ, :])

    nc.compile()
    return nc


_NC_CACHE = None
_RUN_CACHE = None


def _get_runner():
    """Build (once) a cached jitted dispatch for the compiled Bass module.

    Mirrors what bass_utils.run_bass_kernel_spmd does under axon
    (bass2jax.run_bass_via_pjrt), but keeps the jitted executable across
    calls so repeat dispatches skip re-trace/re-lowering.
    """
    global _NC_CACHE, _RUN_CACHE
    if _RUN_CACHE is not None:
        return _RUN_CACHE
    import jax
    from jax.sharding import Mesh, PartitionSpec
    from jax.experimental.shard_map import shard_map
    from concourse.bass2jax import (
        install_neuronx_cc_hook, _bass_exec_p, partition_id_tensor,
    )

    if _NC_CACHE is None:
        _NC_CACHE = build_bass()
    nc = _NC_CACHE
    install_neuronx_cc_hook()
    partition_name = nc.partition_id_tensor.name if nc.partition_id_tensor else None
    in_names, out_names, out_avals, zero_shapes = [], [], [], []
    for alloc in nc.m.functions[0].allocations:
        if not isinstance(alloc, mybir.MemoryLocationSet):
            continue
        name = alloc.memorylocations[0].name
        if alloc.kind == "ExternalInput":
            if name != partition_name:
                in_names.append(name)
        elif alloc.kind == "ExternalOutput":
            out_names.append(name)
            shape = tuple(alloc.tensor_shape)
            dtype = mybir.dt.np(alloc.dtype)
            out_avals.append(jax.core.ShapedArray(shape, dtype))
            zero_shapes.append((shape, dtype))
    n_params = len(in_names)
    n_outs = len(out_avals)
    all_names = list(in_names) + out_names
    if partition_name is not None:
        all_names.append(partition_name)
    donate = tuple(range(n_params, n_params + n_outs))

    def _body(*args):
        operands = list(args)
        if partition_name is not None:
            operands.append(partition_id_tensor())
        outs = _bass_exec_p.bind(
            *operands, out_avals=tuple(out_avals), in_names=tuple(all_names),
            out_names=tuple(out_names), lowering_input_output_aliases=(),
            sim_require_finite=True, sim_require_nnan=True, nc=nc)
        return tuple(outs)

    devices = jax.devices()[:NCORES]
    mesh = Mesh(np.asarray(devices), ("core",))
    sharded = jax.jit(
        shard_map(_body, mesh=mesh,
                  in_specs=(PartitionSpec("core"),) * (n_params + n_outs),
                  out_specs=(PartitionSpec("core"),) * n_outs,
                  check_rep=False),
        donate_argnums=donate, keep_unused=True)
    _RUN_CACHE = (sharded, in_names, out_names, out_avals, zero_shapes)
    return _RUN_CACHE


def _dispatch(in_maps):
    """One full dispatch: host concat -> H2D -> execute -> D2H."""
    import jax
    sharded, in_names, out_names, out_avals, zero_shapes = _get_runner()
    concat_in = [
        np.concatenate([np.asarray(m[name]) for m in in_maps], axis=0)
        for name in in_names
    ]
    concat_zeros = [
        np.zeros((NCORES * s[0], *s[1:]), dt) for s, dt in zero_shapes
    ]
    outs = sharded(*concat_in, *concat_zeros)
    outs = [np.asarray(o) for o in outs]
    return [
        {name: outs[i].reshape(NCORES, *out_avals[i].shape)[c]
         for i, name in enumerate(out_names)}
        for c in range(NCORES)
    ]


def kernel(traversal_lists, adj_matrices, ent_attn, spo_attn,
           ctx_idx_adjusted, roi_cls, roi_mask, weight_on_children):
    in_maps = []
    for k in range(NCORES):
        s = slice(k * BPC, (k + 1) * BPC)
        in_maps.append(_host_prep(
            np.asarray(traversal_lists[s]), np.asarray(adj_matrices[s]),
            np.asarray(ent_attn[s]), np.asarray(spo_attn[s]),
            np.asarray(ctx_idx_adjusted[s]), np.asarray(roi_cls[s]),
            np.asarray(roi_mask[s]), np.asarray(weight_on_children[s]),
        ))
    res = _dispatch(in_maps)
    out = np.empty((BS, NSEQ, NB), dtype=np.float32)
    for k in range(NCORES):
        out[k * BPC:(k + 1) * BPC] = res[k]["ea_out"].reshape(BPC, NSEQ, NB)
    return out
